# revision 12
# baseline (speedup 1.0000x reference)
"""Chebyshev graph convolution (K=3) on 8 Trainium2 NeuronCores.

Strategy (1D destination partitioning, bf16 gather datapath):
- Nodes (destination rows) sharded across 8 cores: core c owns rows
  [c*6250, (c+1)*6250).  Edges partitioned by destination so segment_sum is
  local; per SpMM step the updated node features are AllGather'ed so each
  core can gather arbitrary source rows.
- Node features live in a bf16 table with 128-col (256B) rows, one row per
  node: node (core c, pair j, lane p) -> row (c*128+p)*49 + j.  256B is the
  dma_gather element granularity floor, so bf16 halves the per-edge gather
  bytes vs f32.
- SpMM on-chip: edges grouped by (dest pair j, source table half q) into
  128-edge batches, laid out q-major (all q=0 groups, then all q=1) so each
  step has just two gather spans.  Gathers run in W-batch windows into an
  R-batch SBUF ring; a one-hot selection matrix sel[e,d] = val[e]*(dloc[e]==d)
  is built per window on DVE (bf16), and the TensorEngine accumulates
  psum[d,:] += sel.T @ gathered per (q, j) run; q=0 run seeds S, q=1 adds.
- SWDGE descriptor rings are enlarged (dynamic_dma_scratch_size) so the Pool
  engine's descriptor generation is not backpressured by ring-full stalls
  (the per-queue ring must hold >= 2 gather calls).
- Chebyshev recurrence, U accumulation and the final U @ W + bias run in f32;
  T_k is cast to bf16 only for the writeback + AllGather.
"""

import sys

if "/opt/trn_rl_repo" not in sys.path:
    sys.path.insert(0, "/opt/trn_rl_repo")

import numpy as np
import ml_dtypes

BF16 = ml_dtypes.bfloat16

N_NODES = 50000
D = 96
C = 8  # cores
SH = N_NODES // C  # 6250 rows per core
PAIRS = 49  # ceil(6250/128)
NPAD = C * 128 * PAIRS  # 50176 padded table rows
HALF = NPAD // 2  # 25088
PADC = 128  # table row cols (256B rows in bf16)

last_results = None  # BassKernelResults of the most recent run (for profiling)


def _env_int(name, default):
    import os

    return int(os.environ.get(name, str(default)))


def _row_of_node(g):
    """node id -> table row: (c*128 + p)*49 + j for g = c*6250 + j*128 + p."""
    c, r = g // SH, g % SH
    j, p = r // 128, r % 128
    return (c * 128 + p) * PAIRS + j


def _preprocess(rows, cols, vals, W):
    """Sort/partition edges; q-major padded batch layout.

    Returns (NB, B0, Q0P, TOTB, windows, runs, core_arrays).
    windows: list of (w0, w1, q) batch ranges, each a single dma_gather call.
    runs: list of (q, j, b0, nb) psum accumulation runs in batch order.
    """
    rows = np.asarray(rows).astype(np.int64)
    cols = np.asarray(cols).astype(np.int64)
    vals = np.asarray(vals).astype(np.float32)

    order = np.argsort(rows, kind="stable")
    r_s, c_s, v_s = rows[order], cols[order], vals[order]
    core_bounds = np.searchsorted(r_s, np.arange(C + 1) * SH)

    per_core = []
    counts = np.zeros((C, PAIRS, 2), np.int64)
    for c in range(C):
        s, e = core_bounds[c], core_bounds[c + 1]
        ld = (r_s[s:e] - c * SH).astype(np.int64)
        j = ld // 128
        d128 = (ld % 128).astype(np.float32)
        prow = _row_of_node(c_s[s:e])
        q = (prow >= HALF).astype(np.int64)
        lidx = (prow - q * HALF).astype(np.int64)
        idxmod = _env_int("CHEB_IDXMOD", 0)  # perf probe: clamp index range
        if idxmod:
            lidx = lidx % idxmod
        np.add.at(counts[c], (j, q), 1)
        per_core.append((j, q, d128, lidx, v_s[s:e]))

    NB = -(-counts.max(axis=0) // 128)  # ceil over maxed counts
    NB[:, 0] = np.maximum(NB[:, 0], 1)  # every pair has >=1 batch (q0 seed)

    B0 = np.zeros((PAIRS, 2), np.int64)
    B0[:, 0] = np.cumsum(NB[:, 0]) - NB[:, 0]
    Q0 = int(NB[:, 0].sum())
    Q0P = -(-Q0 // W) * W  # pad q0 span to a window multiple
    B0[:, 1] = Q0P + np.cumsum(NB[:, 1]) - NB[:, 1]
    TOTB = Q0P + int(NB[:, 1].sum())

    windows = []
    for (s0, s1, q) in ((0, Q0P, 0), (Q0P, TOTB, 1)):
        for w0 in range(s0, s1, W):
            windows.append((w0, min(w0 + W, s1), q))

    runs = [(0, j, int(B0[j, 0]), int(NB[j, 0])) for j in range(PAIRS)]
    runs += [(1, j, int(B0[j, 1]), int(NB[j, 1])) for j in range(PAIRS)]

    core_arrays = []
    for c in range(C):
        j, q, d128, lidx, v = per_core[c]
        g_b0 = B0[j, q]  # per-edge group batch offset
        o = np.argsort(g_b0, kind="stable")
        g_sorted = g_b0[o]
        uniq, starts, cnts = np.unique(g_sorted, return_index=True, return_counts=True)
        pos = np.arange(g_sorted.size) - np.repeat(starts, cnts)
        slot = g_sorted * 128 + pos  # global edge slot

        lidx_flat = np.zeros(TOTB * 128, np.int16)
        lane = (slot % 128).astype(np.int64)
        bb = (slot // 128).astype(np.int64)
        lidx_flat[slot] = lidx[o].astype(np.int16)

        # host-precomputed selection matrix: sel[lane, b*128 + dloc] = val
        sel_full = np.zeros((128, TOTB * 128), BF16)
        sel_full[lane, bb * 128 + d128[o].astype(np.int64)] = v[o].astype(BF16)

        # wrapped int16 index tensor: per q span, idx i -> [i%16, i//16]
        widx = np.zeros((16, TOTB * 8), np.int16)
        for (s0, s1) in ((0, Q0P), (Q0P, TOTB)):
            seg = lidx_flat[s0 * 128:s1 * 128]
            n = seg.size
            widx[np.arange(n) % 16, s0 * 8 + np.arange(n) // 16] = seg
        widx = np.tile(widx, (8, 1))
        core_arrays.append((widx, sel_full))

    return NB, B0, Q0P, TOTB, windows, runs, core_arrays


def _build_program(TOTB, windows, runs, W, R):
    import os
    from concourse import bass, bacc, mybir
    import concourse.tile as tile

    no_cc = bool(_env_int("CHEB_NO_CC", 0))
    n_steps = _env_int("CHEB_STEPS", 3)
    no_final = bool(_env_int("CHEB_NO_FINAL", 0))
    nqueues = _env_int("CHEB_QUEUES", 4)
    scratch = _env_int("CHEB_SCRATCH", 49152)

    f32 = mybir.dt.float32
    bf16 = mybir.dt.bfloat16
    nc = bacc.Bacc("TRN2", target_bir_lowering=False, num_devices=C,
                   num_swdge_queues=nqueues,
                   dynamic_dma_scratch_size=scratch)
    gq = [0]  # round-robin gather queue counter

    tbl0 = nc.dram_tensor("tbl0", [NPAD, PADC], bf16, kind="ExternalInput")
    hshc_d = nc.dram_tensor("hshc", [128, PAIRS * D], f32, kind="ExternalInput")
    widx_d = nc.dram_tensor("widx", [128, TOTB * 8], mybir.dt.int16, kind="ExternalInput")
    sel_d = nc.dram_tensor("self", [128, TOTB * 128], bf16, kind="ExternalInput")
    ident_d = nc.dram_tensor("ident", [128, 128], f32, kind="ExternalInput")
    wmat_d = nc.dram_tensor("wmat", [D, D], f32, kind="ExternalInput")
    bias_d = nc.dram_tensor("biasb", [128, D], f32, kind="ExternalInput")
    out_d = nc.dram_tensor("out", [SH, D], f32, kind="ExternalOutput")

    tsh = [nc.dram_tensor(f"tsh{k}", [128, PAIRS * PADC], bf16, kind="Internal")
           for k in (1, 2)]
    tfull = [nc.dram_tensor(f"tfull{k}", [NPAD, PADC], bf16, kind="Internal",
                            addr_space="Shared") for k in (1, 2)]
    rg = [list(range(C))]

    with tile.TileContext(nc) as tc:
        with (
            tc.tile_pool(name="persist", bufs=1) as pp,
            tc.tile_pool(name="up", bufs=2) as up,
            tc.tile_pool(name="psum", bufs=4, space="PSUM") as psp,
            tc.tile_pool(name="psum2", bufs=2, space="PSUM") as psp2,
        ):
            widx_t = pp.tile([128, TOTB * 8], mybir.dt.int16)
            nc.sync.dma_start(out=widx_t[:], in_=widx_d[:, :])
            ident_t = pp.tile([128, 128], f32)
            nc.sync.dma_start(out=ident_t[:], in_=ident_d[:, :])
            wmat_t = pp.tile([D, D], f32)
            nc.sync.dma_start(out=wmat_t[:], in_=wmat_d[:, :])
            bias_t = pp.tile([128, D], f32)
            nc.sync.dma_start(out=bias_t[:], in_=bias_d[:, :])

            Tp = pp.tile([128, PAIRS * D], f32, tag="Tp")
            Tc = pp.tile([128, PAIRS * D], f32, tag="Tc")
            Tc16 = pp.tile([128, PAIRS * PADC], bf16, tag="Tc16")
            U = pp.tile([128, PAIRS * D], f32, tag="U")
            S = pp.tile([128, PAIRS * D], f32, tag="S")
            XG = pp.tile([128, R * 128], bf16, tag="XG")
            SEL = pp.tile([128, R * 128], bf16, tag="SEL")
            xg3 = XG[:].rearrange("p (b f) -> p b f", b=R)
            T163 = Tc16[:].rearrange("p (j f) -> p j f", j=PAIRS)

            nc.gpsimd.memset(Tc16[:], 0.0)  # pad cols stay 0 forever
            nc.sync.dma_start(out=Tp[:], in_=hshc_d[:, :])  # T0 = H
            nc.vector.tensor_copy(out=U[:], in_=Tp[:])

            def spmm(table):
                """S <- spmm over this core's edges, gathering rows of `table`."""
                win_i = 0
                run_i = 0
                ps = [None]

                def emit_window(w0, w1, q):
                    nw = w1 - w0
                    s0 = w0 % R
                    nc.gpsimd.dma_gather(
                        out_ap=xg3[:, s0:s0 + nw, :],
                        in_ap=table[q * HALF:(q + 1) * HALF, :],
                        idxs_ap=widx_t[:, w0 * 8:w1 * 8],
                        num_idxs=nw * 128,
                        num_idxs_reg=nw * 128,
                        elem_size=PADC,
                        queue_num=gq[0] % nqueues,
                        single_packet=bool(_env_int("CHEB_SP", 1)),
                    )
                    # stream the precomputed selection matrix (HWDGE)
                    eng = nc.scalar if gq[0] % 2 else nc.sync
                    eng.dma_start(
                        out=SEL[:, s0 * 128:(s0 + nw) * 128],
                        in_=sel_d[:, w0 * 128:w1 * 128],
                    )
                    gq[0] += 1

                # interleave windows and per-batch matmuls in batch order so
                # program order matches the ring reuse order
                for b in range(TOTB):
                    if win_i < len(windows) and windows[win_i][0] == b:
                        emit_window(*windows[win_i])
                        win_i += 1
                    while run_i < len(runs) and runs[run_i][3] == 0:
                        run_i += 1
                    if run_i >= len(runs) or b < runs[run_i][2]:
                        continue  # padding batch, no consumer
                    q, j, b0, nb = runs[run_i]
                    if b == b0:
                        ps[0] = psp.tile([128, D], f32, tag="ps", name="ps")
                    s = b % R
                    nc.tensor.matmul(
                        out=ps[0][:, :],
                        lhsT=SEL[:, s * 128:(s + 1) * 128],
                        rhs=XG[:, s * 128:s * 128 + D],
                        start=(b == b0),
                        stop=(b == b0 + nb - 1),
                    )
                    if b == b0 + nb - 1:
                        if q == 0:
                            nc.scalar.copy(out=S[:, j * D:(j + 1) * D], in_=ps[0][:])
                        else:
                            nc.vector.tensor_tensor(
                                out=S[:, j * D:(j + 1) * D],
                                in0=S[:, j * D:(j + 1) * D],
                                in1=ps[0][:],
                                op=mybir.AluOpType.add,
                            )
                        run_i += 1

            def writeback(k, src):
                """src (f32) -> Tc16 -> tsh[k] -> AllGather -> tfull[k]."""
                nc.vector.tensor_copy(
                    out=T163[:, :, 0:D],
                    in_=src[:].rearrange("p (j f) -> p j f", j=PAIRS))
                nc.sync.dma_start(out=tsh[k][:, :], in_=Tc16[:])
                nc.gpsimd.collective_compute(
                    "AllGather",
                    mybir.AluOpType.bypass,
                    ins=[tsh[k][:, :]],
                    outs=[tfull[k][:, :]],
                    replica_groups=rg,
                )

            MUL, SUB, ADD = (mybir.AluOpType.mult, mybir.AluOpType.subtract,
                             mybir.AluOpType.add)

            # ---- k=1 : T1 = 2*spmm(H) - T0
            spmm(tbl0)
            nc.vector.scalar_tensor_tensor(
                out=Tc[:], in0=S[:], scalar=2.0, in1=Tp[:], op0=MUL, op1=SUB)
            nc.vector.tensor_tensor(out=U[:], in0=U[:], in1=Tc[:], op=ADD)

            if n_steps >= 2:
                # ---- k=2 : T2 = 2*(2*spmm(T1) - T1) - T0
                if not no_cc:
                    writeback(0, Tc)
                spmm(tbl0 if no_cc else tfull[0])
                nc.vector.scalar_tensor_tensor(
                    out=S[:], in0=S[:], scalar=2.0, in1=Tc[:], op0=MUL, op1=SUB)
                nc.vector.scalar_tensor_tensor(
                    out=Tp[:], in0=S[:], scalar=2.0, in1=Tp[:], op0=MUL, op1=SUB)
                Tp, Tc = Tc, Tp
                nc.vector.tensor_tensor(out=U[:], in0=U[:], in1=Tc[:], op=ADD)

            if n_steps >= 3:
                # ---- k=3 : T3 = 2*(2*spmm(T2) - T2) - T1
                if not no_cc:
                    writeback(1, Tc)
                spmm(tbl0 if no_cc else tfull[1])
                nc.vector.scalar_tensor_tensor(
                    out=S[:], in0=S[:], scalar=2.0, in1=Tc[:], op0=MUL, op1=SUB)
                nc.vector.scalar_tensor_tensor(
                    out=Tp[:], in0=S[:], scalar=2.0, in1=Tp[:], op0=MUL, op1=SUB)
                nc.vector.tensor_tensor(out=U[:], in0=U[:], in1=Tp[:], op=ADD)

            # ---- out = U @ W + bias, written back per pair
            O = S  # S is dead, reuse as output staging
            for j in range(PAIRS) if not no_final else []:
                pt = psp2.tile([128, 128], f32, tag="pt")
                nc.tensor.transpose(
                    out=pt[0:D, :], in_=U[:, j * D:(j + 1) * D], identity=ident_t[:])
                ut = up.tile([128, 128], f32, tag="ut")
                nc.scalar.copy(out=ut[0:D, :], in_=pt[0:D, :])
                po = psp2.tile([128, D], f32, tag="po")
                nc.tensor.matmul(
                    out=po[:], lhsT=ut[0:D, :], rhs=wmat_t[:, :],
                    start=True, stop=True)
                nc.vector.tensor_tensor(
                    out=O[:, j * D:(j + 1) * D], in0=po[:], in1=bias_t[:], op=ADD)
                r1 = min((j + 1) * 128, SH)
                eng = nc.sync if j % 2 == 0 else nc.scalar
                eng.dma_start(
                    out=out_d[j * 128:r1, :],
                    in_=O[0:r1 - j * 128, j * D:(j + 1) * D],
                )

    nc.compile()
    return nc


def kernel(rows, cols, vals, H, W, bias):
    global last_results
    import os
    from concourse.bass_utils import run_bass_kernel_spmd

    H = np.asarray(H).astype(np.float32)
    W = np.asarray(W).astype(np.float32)
    bias = np.asarray(bias).astype(np.float32)

    # NOTE: dma_gather ucode hangs above 1024 indices per call -> W <= 8
    WW = _env_int("CHEB_W", 8)
    R = _env_int("CHEB_RING", 64)
    assert R % WW == 0

    NB, B0, Q0P, TOTB, windows, runs, core_arrays = _preprocess(
        rows, cols, vals, WW)
    nc = _build_program(TOTB, windows, runs, WW, R)

    # bf16 node table [NPAD, 128] in (c*128+p)*49+j order
    tbl = np.zeros((NPAD, PADC), BF16)
    tbl[_row_of_node(np.arange(N_NODES)), :D] = H.astype(BF16)

    ident = np.eye(128, dtype=np.float32)
    biasb = np.broadcast_to(bias, (128, D)).copy()

    in_maps = []
    for c in range(C):
        widx, sel_full = core_arrays[c]
        # hshc: compact [128, 49*96] partition-major layout of this core's shard
        hshc = np.zeros((128, PAIRS, D), np.float32)
        hrows = H[c * SH:(c + 1) * SH]
        for j in range(PAIRS):
            r0, r1 = j * 128, min((j + 1) * 128, SH)
            hshc[0:r1 - r0, j, :] = hrows[r0:r1]
        in_maps.append({
            "tbl0": tbl,
            "hshc": hshc.reshape(128, PAIRS * D),
            "widx": widx,
            "self": sel_full,
            "ident": ident,
            "wmat": W,
            "biasb": biasb,
        })

    res = run_bass_kernel_spmd(
        nc, in_maps, core_ids=list(range(C)),
        trace=bool(_env_int("CHEB_TRACE", 0)),
    )
    last_results = res
    return np.concatenate([res.results[c]["out"] for c in range(C)], axis=0)


# revision 21
# speedup vs baseline: 1.1097x; 1.1097x over previous
"""Chebyshev graph convolution (K=3) on 8 Trainium2 NeuronCores.

Strategy (1D destination partitioning, bf16 gather datapath):
- Nodes (destination rows) sharded across 8 cores: core c owns rows
  [c*6250, (c+1)*6250).  Edges partitioned by destination so segment_sum is
  local; per SpMM step the updated node features are AllGather'ed so each
  core can gather arbitrary source rows.
- Node features live in a bf16 table with 128-col (256B) rows, one row per
  node: node (core c, pair j, lane p) -> row (c*128+p)*49 + j.  256B is the
  dma_gather element granularity floor, so bf16 halves the per-edge gather
  bytes vs f32.
- SpMM on-chip: edges grouped by (dest pair j, source table half q) into
  128-edge batches, laid out q-major (all q=0 groups, then all q=1) so each
  step has just two gather spans.  Gathers run in W-batch windows into an
  R-batch SBUF ring; a one-hot selection matrix sel[e,d] = val[e]*(dloc[e]==d)
  is built per window on DVE (bf16), and the TensorEngine accumulates
  psum[d,:] += sel.T @ gathered per (q, j) run; q=0 run seeds S, q=1 adds.
- SWDGE descriptor rings are enlarged (dynamic_dma_scratch_size) so the Pool
  engine's descriptor generation is not backpressured by ring-full stalls
  (the per-queue ring must hold >= 2 gather calls).
- Chebyshev recurrence, U accumulation and the final U @ W + bias run in f32;
  T_k is cast to bf16 only for the writeback + AllGather.
"""

import sys

if "/opt/trn_rl_repo" not in sys.path:
    sys.path.insert(0, "/opt/trn_rl_repo")

import numpy as np
import ml_dtypes

BF16 = ml_dtypes.bfloat16

N_NODES = 50000
D = 96
C = 8  # cores
SH = N_NODES // C  # 6250 rows per core
PAIRS = 49  # ceil(6250/128)
NPAD = C * 128 * PAIRS  # 50176 padded table rows
HALF = NPAD // 2  # 25088
PADC = 128  # table row cols (256B rows in bf16)

last_results = None  # BassKernelResults of the most recent run (for profiling)


def _env_int(name, default):
    import os

    return int(os.environ.get(name, str(default)))


def _row_of_node(g):
    """node id -> table row: (c*128 + p)*49 + j for g = c*6250 + j*128 + p."""
    c, r = g // SH, g % SH
    j, p = r // 128, r % 128
    return (c * 128 + p) * PAIRS + j


def _preprocess(rows, cols, vals, W):
    """Sort/partition edges; q-major padded batch layout.

    Returns (NB, B0, Q0P, TOTB, windows, runs, core_arrays).
    windows: list of (w0, w1, q) batch ranges, each a single dma_gather call.
    runs: list of (q, j, b0, nb) psum accumulation runs in batch order.
    """
    rows = np.asarray(rows).astype(np.int64)
    cols = np.asarray(cols).astype(np.int64)
    vals = np.asarray(vals).astype(np.float32)

    order = np.argsort(rows, kind="stable")
    r_s, c_s, v_s = rows[order], cols[order], vals[order]
    core_bounds = np.searchsorted(r_s, np.arange(C + 1) * SH)

    per_core = []
    counts = np.zeros((C, PAIRS, 2), np.int64)
    for c in range(C):
        s, e = core_bounds[c], core_bounds[c + 1]
        ld = (r_s[s:e] - c * SH).astype(np.int64)
        j = ld // 128
        d128 = (ld % 128).astype(np.float32)
        prow = _row_of_node(c_s[s:e])
        q = (prow >= HALF).astype(np.int64)
        lidx = (prow - q * HALF).astype(np.int64)
        idxmod = _env_int("CHEB_IDXMOD", 0)  # perf probe: clamp index range
        if idxmod:
            lidx = lidx % idxmod
        np.add.at(counts[c], (j, q), 1)
        per_core.append((j, q, d128, lidx, v_s[s:e]))

    NB = -(-counts.max(axis=0) // 128)  # ceil over maxed counts
    NB[:, 0] = np.maximum(NB[:, 0], 1)  # every pair has >=1 batch (q0 seed)

    B0 = np.zeros((PAIRS, 2), np.int64)
    B0[:, 0] = np.cumsum(NB[:, 0]) - NB[:, 0]
    Q0 = int(NB[:, 0].sum())
    Q0P = -(-Q0 // W) * W  # pad q0 span to a window multiple
    B0[:, 1] = Q0P + np.cumsum(NB[:, 1]) - NB[:, 1]
    TOTB = Q0P + int(NB[:, 1].sum())

    windows = []
    for (s0, s1, q) in ((0, Q0P, 0), (Q0P, TOTB, 1)):
        for w0 in range(s0, s1, W):
            windows.append((w0, min(w0 + W, s1), q))

    runs = [(0, j, int(B0[j, 0]), int(NB[j, 0])) for j in range(PAIRS)]
    runs += [(1, j, int(B0[j, 1]), int(NB[j, 1])) for j in range(PAIRS)]

    core_arrays = []
    for c in range(C):
        j, q, d128, lidx, v = per_core[c]
        g_b0 = B0[j, q]  # per-edge group batch offset
        o = np.argsort(g_b0, kind="stable")
        g_sorted = g_b0[o]
        uniq, starts, cnts = np.unique(g_sorted, return_index=True, return_counts=True)
        pos = np.arange(g_sorted.size) - np.repeat(starts, cnts)
        slot = g_sorted * 128 + pos  # global edge slot

        lidx_flat = np.zeros(TOTB * 128, np.int16)
        dloc_col = np.zeros((128, TOTB), np.float32)
        vals_col = np.zeros((128, TOTB), np.float32)
        lane = (slot % 128).astype(np.int64)
        bb = (slot // 128).astype(np.int64)
        lidx_flat[slot] = lidx[o].astype(np.int16)
        dloc_col[lane, bb] = d128[o]
        vals_col[lane, bb] = v[o]

        # wrapped int16 index tensor: per q span, idx i -> [i%16, i//16]
        widx = np.zeros((16, TOTB * 8), np.int16)
        for (s0, s1) in ((0, Q0P), (Q0P, TOTB)):
            seg = lidx_flat[s0 * 128:s1 * 128]
            n = seg.size
            widx[np.arange(n) % 16, s0 * 8 + np.arange(n) // 16] = seg
        widx = np.tile(widx, (8, 1))
        core_arrays.append(
            (widx, dloc_col.astype(BF16), vals_col.astype(BF16)))

    return NB, B0, Q0P, TOTB, windows, runs, core_arrays


def _build_program(TOTB, windows, runs, W, R):
    import os
    from concourse import bass, bacc, mybir
    import concourse.tile as tile

    no_cc = bool(_env_int("CHEB_NO_CC", 0))
    n_steps = _env_int("CHEB_STEPS", 3)
    no_final = bool(_env_int("CHEB_NO_FINAL", 0))
    nqueues = _env_int("CHEB_QUEUES", 4)
    scratch = _env_int("CHEB_SCRATCH", 40960)

    f32 = mybir.dt.float32
    bf16 = mybir.dt.bfloat16
    nc = bacc.Bacc("TRN2", target_bir_lowering=False, num_devices=C,
                   num_swdge_queues=nqueues,
                   dynamic_dma_scratch_size=scratch)
    gq = [0]  # round-robin gather queue counter

    tbl0 = nc.dram_tensor("tbl0", [NPAD, PADC], bf16, kind="ExternalInput")
    hshc_d = nc.dram_tensor("hshc", [128, PAIRS * D], f32, kind="ExternalInput")
    widx_d = nc.dram_tensor("widx", [128, TOTB * 8], mybir.dt.int16, kind="ExternalInput")
    dloc_d = nc.dram_tensor("dloc", [128, TOTB], bf16, kind="ExternalInput")
    wval_d = nc.dram_tensor("wval", [128, TOTB], bf16, kind="ExternalInput")
    iota_d = nc.dram_tensor("iota128", [128, 128], bf16, kind="ExternalInput")
    ident_d = nc.dram_tensor("ident", [128, 128], f32, kind="ExternalInput")
    wmat_d = nc.dram_tensor("wmat", [D, D], f32, kind="ExternalInput")
    bias_d = nc.dram_tensor("biasb", [128, D], f32, kind="ExternalInput")
    out_d = nc.dram_tensor("out", [SH, D], f32, kind="ExternalOutput")

    tsh = [nc.dram_tensor(f"tsh{k}", [128, PAIRS * PADC], bf16, kind="Internal")
           for k in (1, 2)]
    tfull = [nc.dram_tensor(f"tfull{k}", [NPAD, PADC], bf16, kind="Internal",
                            addr_space="Shared") for k in (1, 2)]
    rg = [list(range(C))]

    with tile.TileContext(nc) as tc:
        with (
            tc.tile_pool(name="persist", bufs=1) as pp,
            tc.tile_pool(name="up", bufs=2) as up,
            tc.tile_pool(name="psum", bufs=4, space="PSUM") as psp,
            tc.tile_pool(name="psum2", bufs=2, space="PSUM") as psp2,
        ):
            widx_t = pp.tile([128, TOTB * 8], mybir.dt.int16)
            nc.sync.dma_start(out=widx_t[:], in_=widx_d[:, :])
            dloc_t = pp.tile([128, TOTB], bf16)
            nc.sync.dma_start(out=dloc_t[:], in_=dloc_d[:, :])
            wval_t = pp.tile([128, TOTB], bf16)
            nc.sync.dma_start(out=wval_t[:], in_=wval_d[:, :])
            iota_t = pp.tile([128, 128], bf16)
            nc.sync.dma_start(out=iota_t[:], in_=iota_d[:, :])
            ident_t = pp.tile([128, 128], f32)
            nc.sync.dma_start(out=ident_t[:], in_=ident_d[:, :])
            wmat_t = pp.tile([D, D], f32)
            nc.sync.dma_start(out=wmat_t[:], in_=wmat_d[:, :])
            bias_t = pp.tile([128, D], f32)
            nc.sync.dma_start(out=bias_t[:], in_=bias_d[:, :])

            Tp = pp.tile([128, PAIRS * D], f32, tag="Tp")
            Tc = pp.tile([128, PAIRS * D], f32, tag="Tc")
            Tc16 = pp.tile([128, PAIRS * PADC], bf16, tag="Tc16")
            U = pp.tile([128, PAIRS * D], f32, tag="U")
            S = pp.tile([128, PAIRS * D], f32, tag="S")
            S1 = pp.tile([128, PAIRS * D], f32, tag="S1")
            XG = pp.tile([128, R * 128], bf16, tag="XG")
            SEL = pp.tile([128, R * 128], bf16, tag="SEL")
            xg3 = XG[:].rearrange("p (b f) -> p b f", b=R)
            sel3 = SEL[:].rearrange("p (b f) -> p b f", b=R)
            T163 = Tc16[:].rearrange("p (j f) -> p j f", j=PAIRS)

            nc.gpsimd.memset(Tc16[:], 0.0)  # pad cols stay 0 forever
            nc.gpsimd.memset(S1[:], 0.0)  # pairs with no q1 batches stay 0
            nc.sync.dma_start(out=Tp[:], in_=hshc_d[:, :])  # T0 = H
            nc.vector.tensor_copy(out=U[:], in_=Tp[:])

            def spmm(table):
                """S <- spmm over this core's edges, gathering rows of `table`."""
                win_i = 0
                run_i = 0
                ps = [None]

                def emit_window(w0, w1, q):
                    nw = w1 - w0
                    s0 = w0 % R
                    nc.gpsimd.dma_gather(
                        out_ap=xg3[:, s0:s0 + nw, :],
                        in_ap=table[q * HALF:(q + 1) * HALF, :],
                        idxs_ap=widx_t[:, w0 * 8:w1 * 8],
                        num_idxs=nw * 128,
                        num_idxs_reg=nw * 128,
                        elem_size=PADC,
                        queue_num=gq[0] % nqueues,
                        single_packet=bool(_env_int("CHEB_SP", 1)),
                    )
                    gq[0] += 1
                    # sel build on DVE (feed-forward: no mid-step DVE consumers)
                    iota_b = bass.AP(
                        iota_t[:].tensor, iota_t[:].offset,
                        [iota_t[:].ap[0], [0, nw], [1, 128]],
                    )
                    nc.vector.tensor_tensor(
                        out=sel3[:, s0:s0 + nw, :],
                        in0=dloc_t[:, w0:w1].to_broadcast([128, nw, 128]),
                        in1=iota_b,
                        op=mybir.AluOpType.is_equal,
                    )
                    nc.vector.tensor_tensor(
                        out=sel3[:, s0:s0 + nw, :],
                        in0=sel3[:, s0:s0 + nw, :],
                        in1=wval_t[:, w0:w1].to_broadcast([128, nw, 128]),
                        op=mybir.AluOpType.mult,
                    )

                # interleave windows and per-batch matmuls in batch order so
                # program order matches the ring reuse order
                for b in range(TOTB):
                    if win_i < len(windows) and windows[win_i][0] == b:
                        emit_window(*windows[win_i])
                        win_i += 1
                    while run_i < len(runs) and runs[run_i][3] == 0:
                        run_i += 1
                    if run_i >= len(runs) or b < runs[run_i][2]:
                        continue  # padding batch, no consumer
                    q, j, b0, nb = runs[run_i]
                    if b == b0:
                        ps[0] = psp.tile([128, D], f32, tag="ps", name="ps")
                    s = b % R
                    nc.tensor.matmul(
                        out=ps[0][:, :],
                        lhsT=SEL[:, s * 128:(s + 1) * 128],
                        rhs=XG[:, s * 128:s * 128 + D],
                        start=(b == b0),
                        stop=(b == b0 + nb - 1),
                    )
                    if b == b0 + nb - 1:
                        # both q0 and q1 land via the Scalar engine so the DVE
                        # stream stays free of psum round-trips; S += S1 merges
                        # once at step end
                        dst = S if q == 0 else S1
                        nc.scalar.copy(out=dst[:, j * D:(j + 1) * D], in_=ps[0][:])
                        run_i += 1

            def writeback(k, src):
                """src (f32) -> Tc16 -> tsh[k] -> AllGather -> tfull[k]."""
                nc.vector.tensor_copy(
                    out=T163[:, :, 0:D],
                    in_=src[:].rearrange("p (j f) -> p j f", j=PAIRS))
                nc.sync.dma_start(out=tsh[k][:, :], in_=Tc16[:])
                nc.gpsimd.collective_compute(
                    "AllGather",
                    mybir.AluOpType.bypass,
                    ins=[tsh[k][:, :]],
                    outs=[tfull[k][:, :]],
                    replica_groups=rg,
                )

            MUL, SUB, ADD = (mybir.AluOpType.mult, mybir.AluOpType.subtract,
                             mybir.AluOpType.add)

            def merge_s():  # S += S1 (q1 partial sums), once per step
                nc.vector.tensor_tensor(out=S[:], in0=S[:], in1=S1[:], op=ADD)

            # ---- k=1 : T1 = 2*spmm(H) - T0
            spmm(tbl0)
            merge_s()
            nc.vector.scalar_tensor_tensor(
                out=Tc[:], in0=S[:], scalar=2.0, in1=Tp[:], op0=MUL, op1=SUB)
            nc.vector.tensor_tensor(out=U[:], in0=U[:], in1=Tc[:], op=ADD)

            if n_steps >= 2:
                # ---- k=2 : T2 = 2*(2*spmm(T1) - T1) - T0
                if not no_cc:
                    writeback(0, Tc)
                spmm(tbl0 if no_cc else tfull[0])
                merge_s()
                nc.vector.scalar_tensor_tensor(
                    out=S[:], in0=S[:], scalar=2.0, in1=Tc[:], op0=MUL, op1=SUB)
                nc.vector.scalar_tensor_tensor(
                    out=Tp[:], in0=S[:], scalar=2.0, in1=Tp[:], op0=MUL, op1=SUB)
                Tp, Tc = Tc, Tp
                nc.vector.tensor_tensor(out=U[:], in0=U[:], in1=Tc[:], op=ADD)

            if n_steps >= 3:
                # ---- k=3 : T3 = 2*(2*spmm(T2) - T2) - T1
                if not no_cc:
                    writeback(1, Tc)
                spmm(tbl0 if no_cc else tfull[1])
                merge_s()
                nc.vector.scalar_tensor_tensor(
                    out=S[:], in0=S[:], scalar=2.0, in1=Tc[:], op0=MUL, op1=SUB)
                nc.vector.scalar_tensor_tensor(
                    out=Tp[:], in0=S[:], scalar=2.0, in1=Tp[:], op0=MUL, op1=SUB)
                nc.vector.tensor_tensor(out=U[:], in0=U[:], in1=Tp[:], op=ADD)

            # ---- out = U @ W + bias, written back per pair
            O = S  # S is dead, reuse as output staging
            for j in range(PAIRS) if not no_final else []:
                pt = psp2.tile([128, 128], f32, tag="pt")
                nc.tensor.transpose(
                    out=pt[0:D, :], in_=U[:, j * D:(j + 1) * D], identity=ident_t[:])
                ut = up.tile([128, 128], f32, tag="ut")
                nc.scalar.copy(out=ut[0:D, :], in_=pt[0:D, :])
                po = psp2.tile([128, D], f32, tag="po")
                nc.tensor.matmul(
                    out=po[:], lhsT=ut[0:D, :], rhs=wmat_t[:, :],
                    start=True, stop=True)
                nc.vector.tensor_tensor(
                    out=O[:, j * D:(j + 1) * D], in0=po[:], in1=bias_t[:], op=ADD)
                r1 = min((j + 1) * 128, SH)
                eng = nc.sync if j % 2 == 0 else nc.scalar
                eng.dma_start(
                    out=out_d[j * 128:r1, :],
                    in_=O[0:r1 - j * 128, j * D:(j + 1) * D],
                )

    nc.compile()
    return nc


def kernel(rows, cols, vals, H, W, bias):
    global last_results
    import os
    from concourse.bass_utils import run_bass_kernel_spmd

    H = np.asarray(H).astype(np.float32)
    W = np.asarray(W).astype(np.float32)
    bias = np.asarray(bias).astype(np.float32)

    # NOTE: dma_gather ucode hangs above 1024 indices per call -> W <= 8
    WW = _env_int("CHEB_W", 8)
    R = _env_int("CHEB_RING", 64)
    assert R % WW == 0

    NB, B0, Q0P, TOTB, windows, runs, core_arrays = _preprocess(
        rows, cols, vals, WW)
    nc = _build_program(TOTB, windows, runs, WW, R)

    # bf16 node table [NPAD, 128] in (c*128+p)*49+j order
    tbl = np.zeros((NPAD, PADC), BF16)
    tbl[_row_of_node(np.arange(N_NODES)), :D] = H.astype(BF16)

    iota128 = np.broadcast_to(np.arange(128, dtype=np.float32), (128, 128))
    iota128 = iota128.astype(BF16)
    ident = np.eye(128, dtype=np.float32)
    biasb = np.broadcast_to(bias, (128, D)).copy()

    in_maps = []
    for c in range(C):
        widx, dloc_col, vals_col = core_arrays[c]
        # hshc: compact [128, 49*96] partition-major layout of this core's shard
        hshc = np.zeros((128, PAIRS, D), np.float32)
        hrows = H[c * SH:(c + 1) * SH]
        for j in range(PAIRS):
            r0, r1 = j * 128, min((j + 1) * 128, SH)
            hshc[0:r1 - r0, j, :] = hrows[r0:r1]
        in_maps.append({
            "tbl0": tbl,
            "hshc": hshc.reshape(128, PAIRS * D),
            "widx": widx,
            "dloc": dloc_col,
            "wval": vals_col,
            "iota128": iota128,
            "ident": ident,
            "wmat": W,
            "biasb": biasb,
        })

    res = run_bass_kernel_spmd(
        nc, in_maps, core_ids=list(range(C)),
        trace=bool(_env_int("CHEB_TRACE", 0)),
    )
    last_results = res
    return np.concatenate([res.results[c]["out"] for c in range(C)], axis=0)


# revision 22
# speedup vs baseline: 1.2978x; 1.1695x over previous
"""Chebyshev graph convolution (K=3) on 8 Trainium2 NeuronCores.

Strategy (1D destination partitioning, bf16 gather datapath):
- Nodes (destination rows) sharded across 8 cores: core c owns rows
  [c*6250, (c+1)*6250).  Edges partitioned by destination so segment_sum is
  local; per SpMM step the updated node features are AllGather'ed so each
  core can gather arbitrary source rows.
- Node features live in bf16 tables with 128-col (256B) rows, one row per
  node.  256B is the dma_gather element granularity floor, so bf16 halves
  the per-edge gather bytes vs f32.  The table is split in TWO halves by
  pair-range within each shard (pairs 0..24 -> table A, 25..48 -> table B):
  node (c, j, p) -> rowA (c*128+p)*25 + j  or  rowB (c*128+p)*24 + (j-25).
  Each half is AllGather'ed separately, and the A-half collective + its
  recurrence/cast run EARLY (as soon as destination pairs 0..24 finish),
  overlapping the B-half compute; the next step's A-sourced gathers wait
  only on the A collective.  This pipelines the step boundary.
- SpMM on-chip: edges grouped by (dest pair j, source half q) into 128-edge
  batches, laid out q-major.  Gathers run in 8-batch windows (1024 indices,
  the dma_gather ucode limit) into an R-batch SBUF ring; a one-hot selection
  matrix sel[e,d] = val[e]*(dloc[e]==d) is built per window on DVE (bf16),
  and the TensorEngine accumulates psum[d,:] += sel.T @ gathered per (q, j)
  run.  q=0 runs seed S via the Scalar engine, q=1 runs land in a scratch S1
  (Scalar as well) merged once per half -- the mid-step DVE stream stays
  free of psum round-trips so sel builds never stall the gather pipeline.
- SWDGE descriptor rings are enlarged (dynamic_dma_scratch_size=48K) so >=2
  gather calls fit per queue ring.
- Chebyshev recurrence, U accumulation and the final U @ W + bias run in
  f32; T_k is cast to bf16 only for the writeback + AllGather.
"""

import sys

if "/opt/trn_rl_repo" not in sys.path:
    sys.path.insert(0, "/opt/trn_rl_repo")

import numpy as np
import ml_dtypes

BF16 = ml_dtypes.bfloat16

N_NODES = 50000
D = 96
C = 8  # cores
SH = N_NODES // C  # 6250 rows per core
PAIRS = 49  # ceil(6250/128)
JH = 25  # pairs in table half A
JB = PAIRS - JH  # 24 pairs in half B
ROWSA = C * 128 * JH  # 25600
ROWSB = C * 128 * JB  # 24576
NPAD = ROWSA + ROWSB  # 50176
PADC = 128  # table row cols (256B rows in bf16)

last_results = None  # BassKernelResults of the most recent run (for profiling)


def _env_int(name, default):
    import os

    return int(os.environ.get(name, str(default)))


def _row_q_of_node(g):
    """node id -> (q, row-within-half)."""
    g = np.asarray(g)
    c, r = g // SH, g % SH
    j, p = r // 128, r % 128
    q = (j >= JH).astype(np.int64)
    row = np.where(q == 0, (c * 128 + p) * JH + j,
                   (c * 128 + p) * JB + (j - JH))
    return q, row


def _preprocess(rows, cols, vals, W):
    """Sort/partition edges; q-major padded batch layout.

    Returns (TOTB, windows, runs, core_arrays).
    windows: list of (w0, w1, q) batch ranges, each a single dma_gather call.
    runs: list of (q, j, b0, nb) psum accumulation runs in batch order.
    """
    rows = np.asarray(rows).astype(np.int64)
    cols = np.asarray(cols).astype(np.int64)
    vals = np.asarray(vals).astype(np.float32)

    order = np.argsort(rows, kind="stable")
    r_s, c_s, v_s = rows[order], cols[order], vals[order]
    core_bounds = np.searchsorted(r_s, np.arange(C + 1) * SH)

    per_core = []
    counts = np.zeros((C, PAIRS, 2), np.int64)
    for c in range(C):
        s, e = core_bounds[c], core_bounds[c + 1]
        ld = (r_s[s:e] - c * SH).astype(np.int64)
        j = ld // 128
        d128 = (ld % 128).astype(np.float32)
        q, lidx = _row_q_of_node(c_s[s:e])
        idxmod = _env_int("CHEB_IDXMOD", 0)  # perf probe: clamp index range
        if idxmod:
            lidx = lidx % idxmod
        np.add.at(counts[c], (j, q), 1)
        per_core.append((j, q, d128, lidx, v_s[s:e]))

    NB = -(-counts.max(axis=0) // 128)  # ceil over maxed counts
    NB[:, 0] = np.maximum(NB[:, 0], 1)  # every pair has >=1 batch (q0 seed)

    B0 = np.zeros((PAIRS, 2), np.int64)
    B0[:, 0] = np.cumsum(NB[:, 0]) - NB[:, 0]
    Q0 = int(NB[:, 0].sum())
    Q0P = -(-Q0 // W) * W  # pad q0 span to a window multiple
    B0[:, 1] = Q0P + np.cumsum(NB[:, 1]) - NB[:, 1]
    TOTB = Q0P + int(NB[:, 1].sum())

    windows = []
    for (s0, s1, q) in ((0, Q0P, 0), (Q0P, TOTB, 1)):
        for w0 in range(s0, s1, W):
            windows.append((w0, min(w0 + W, s1), q))

    runs = [(0, j, int(B0[j, 0]), int(NB[j, 0])) for j in range(PAIRS)]
    runs += [(1, j, int(B0[j, 1]), int(NB[j, 1])) for j in range(PAIRS)]

    core_arrays = []
    for c in range(C):
        j, q, d128, lidx, v = per_core[c]
        g_b0 = B0[j, q]  # per-edge group batch offset
        o = np.argsort(g_b0, kind="stable")
        g_sorted = g_b0[o]
        uniq, starts, cnts = np.unique(g_sorted, return_index=True, return_counts=True)
        pos = np.arange(g_sorted.size) - np.repeat(starts, cnts)
        slot = g_sorted * 128 + pos  # global edge slot

        lidx_flat = np.zeros(TOTB * 128, np.int16)
        dloc_col = np.zeros((128, TOTB), np.float32)
        vals_col = np.zeros((128, TOTB), np.float32)
        lane = (slot % 128).astype(np.int64)
        bb = (slot // 128).astype(np.int64)
        lidx_flat[slot] = lidx[o].astype(np.int16)
        dloc_col[lane, bb] = d128[o]
        vals_col[lane, bb] = v[o]

        # wrapped int16 index tensor: per q span, idx i -> [i%16, i//16]
        widx = np.zeros((16, TOTB * 8), np.int16)
        for (s0, s1) in ((0, Q0P), (Q0P, TOTB)):
            seg = lidx_flat[s0 * 128:s1 * 128]
            n = seg.size
            widx[np.arange(n) % 16, s0 * 8 + np.arange(n) // 16] = seg
        widx = np.tile(widx, (8, 1))
        core_arrays.append(
            (widx, dloc_col.astype(BF16), vals_col.astype(BF16)))

    return TOTB, windows, runs, core_arrays


def _build_program(TOTB, windows, runs, W, R):
    from concourse import bass, bacc, mybir
    import concourse.tile as tile

    no_cc = bool(_env_int("CHEB_NO_CC", 0))
    n_steps = _env_int("CHEB_STEPS", 3)
    no_final = bool(_env_int("CHEB_NO_FINAL", 0))
    nqueues = _env_int("CHEB_QUEUES", 4)
    scratch = _env_int("CHEB_SCRATCH", 49152)

    f32 = mybir.dt.float32
    bf16 = mybir.dt.bfloat16
    MUL, SUB, ADD = (mybir.AluOpType.mult, mybir.AluOpType.subtract,
                     mybir.AluOpType.add)
    nc = bacc.Bacc("TRN2", target_bir_lowering=False, num_devices=C,
                   num_swdge_queues=nqueues,
                   dynamic_dma_scratch_size=scratch)
    gq = [0]  # round-robin gather queue counter

    tbl0 = nc.dram_tensor("tbl0", [NPAD, PADC], bf16, kind="ExternalInput")
    hshc_d = nc.dram_tensor("hshc", [128, PAIRS * D], f32, kind="ExternalInput")
    widx_d = nc.dram_tensor("widx", [128, TOTB * 8], mybir.dt.int16, kind="ExternalInput")
    dloc_d = nc.dram_tensor("dloc", [128, TOTB], bf16, kind="ExternalInput")
    wval_d = nc.dram_tensor("wval", [128, TOTB], bf16, kind="ExternalInput")
    iota_d = nc.dram_tensor("iota128", [128, 128], bf16, kind="ExternalInput")
    ident_d = nc.dram_tensor("ident", [128, 128], f32, kind="ExternalInput")
    wmat_d = nc.dram_tensor("wmat", [D, D], f32, kind="ExternalInput")
    bias_d = nc.dram_tensor("biasb", [128, D], f32, kind="ExternalInput")
    out_d = nc.dram_tensor("out", [SH, D], f32, kind="ExternalOutput")

    tshA = [nc.dram_tensor(f"tshA{k}", [128, JH * PADC], bf16, kind="Internal")
            for k in (1, 2)]
    tshB = [nc.dram_tensor(f"tshB{k}", [128, JB * PADC], bf16, kind="Internal")
            for k in (1, 2)]
    tfullA = [nc.dram_tensor(f"tfullA{k}", [ROWSA, PADC], bf16, kind="Internal",
                             addr_space="Shared") for k in (1, 2)]
    tfullB = [nc.dram_tensor(f"tfullB{k}", [ROWSB, PADC], bf16, kind="Internal",
                             addr_space="Shared") for k in (1, 2)]
    rg = [list(range(C))]

    # index of the run after which the A half (dest pairs 0..JH-1) is complete
    half_run_i = max(i for i, (q, j, b0, nb) in enumerate(runs)
                     if q == 1 and j < JH and nb > 0)

    with tile.TileContext(nc) as tc:
        with (
            tc.tile_pool(name="persist", bufs=1) as pp,
            tc.tile_pool(name="up", bufs=2) as up,
            tc.tile_pool(name="psum", bufs=4, space="PSUM") as psp,
            tc.tile_pool(name="psum2", bufs=2, space="PSUM") as psp2,
        ):
            widx_t = pp.tile([128, TOTB * 8], mybir.dt.int16)
            nc.sync.dma_start(out=widx_t[:], in_=widx_d[:, :])
            dloc_t = pp.tile([128, TOTB], bf16)
            nc.sync.dma_start(out=dloc_t[:], in_=dloc_d[:, :])
            wval_t = pp.tile([128, TOTB], bf16)
            nc.sync.dma_start(out=wval_t[:], in_=wval_d[:, :])
            iota_t = pp.tile([128, 128], bf16)
            nc.sync.dma_start(out=iota_t[:], in_=iota_d[:, :])
            ident_t = pp.tile([128, 128], f32)
            nc.sync.dma_start(out=ident_t[:], in_=ident_d[:, :])
            wmat_t = pp.tile([D, D], f32)
            nc.sync.dma_start(out=wmat_t[:], in_=wmat_d[:, :])
            bias_t = pp.tile([128, D], f32)
            nc.sync.dma_start(out=bias_t[:], in_=bias_d[:, :])

            Tp = pp.tile([128, PAIRS * D], f32, tag="Tp")
            Tc = pp.tile([128, PAIRS * D], f32, tag="Tc")
            Tc16 = pp.tile([128, PAIRS * PADC], bf16, tag="Tc16")
            U = pp.tile([128, PAIRS * D], f32, tag="U")
            S = pp.tile([128, PAIRS * D], f32, tag="S")
            S1 = pp.tile([128, PAIRS * D], f32, tag="S1")
            XG = pp.tile([128, R * 128], bf16, tag="XG")
            SEL = pp.tile([128, R * 128], bf16, tag="SEL")
            xg3 = XG[:].rearrange("p (b f) -> p b f", b=R)
            sel3 = SEL[:].rearrange("p (b f) -> p b f", b=R)
            T163 = Tc16[:].rearrange("p (j f) -> p j f", j=PAIRS)

            nc.gpsimd.memset(Tc16[:], 0.0)  # pad cols stay 0 forever
            nc.gpsimd.memset(S1[:], 0.0)  # pairs with no q1 batches stay 0
            nc.sync.dma_start(out=Tp[:], in_=hshc_d[:, :])  # T0 = H
            nc.vector.tensor_copy(out=U[:], in_=Tp[:])

            def spmm(tableA, tableB, half_cb, end_cb):
                """S/S1 <- spmm partials; fires half_cb after dest pairs
                [0, JH) complete and end_cb after all pairs."""
                win_i = 0
                run_i = 0
                ps = [None]

                def emit_window(w0, w1, q):
                    nw = w1 - w0
                    s0 = w0 % R
                    nc.gpsimd.dma_gather(
                        out_ap=xg3[:, s0:s0 + nw, :],
                        in_ap=(tableB if q else tableA)[:, :],
                        idxs_ap=widx_t[:, w0 * 8:w1 * 8],
                        num_idxs=nw * 128,
                        num_idxs_reg=nw * 128,
                        elem_size=PADC,
                        queue_num=gq[0] % nqueues,
                    )
                    gq[0] += 1
                    iota_b = bass.AP(
                        iota_t[:].tensor, iota_t[:].offset,
                        [iota_t[:].ap[0], [0, nw], [1, 128]],
                    )
                    nc.vector.tensor_tensor(
                        out=sel3[:, s0:s0 + nw, :],
                        in0=dloc_t[:, w0:w1].to_broadcast([128, nw, 128]),
                        in1=iota_b,
                        op=mybir.AluOpType.is_equal,
                    )
                    nc.vector.tensor_tensor(
                        out=sel3[:, s0:s0 + nw, :],
                        in0=sel3[:, s0:s0 + nw, :],
                        in1=wval_t[:, w0:w1].to_broadcast([128, nw, 128]),
                        op=mybir.AluOpType.mult,
                    )

                # interleave windows and per-batch matmuls in batch order so
                # program order matches the ring reuse order
                for b in range(TOTB):
                    if win_i < len(windows) and windows[win_i][0] == b:
                        emit_window(*windows[win_i])
                        win_i += 1
                    while run_i < len(runs) and runs[run_i][3] == 0:
                        run_i += 1
                    if run_i >= len(runs) or b < runs[run_i][2]:
                        continue  # padding batch, no consumer
                    q, j, b0, nb = runs[run_i]
                    if b == b0:
                        ps[0] = psp.tile([128, D], f32, tag="ps", name="ps")
                    s = b % R
                    nc.tensor.matmul(
                        out=ps[0][:, :],
                        lhsT=SEL[:, s * 128:(s + 1) * 128],
                        rhs=XG[:, s * 128:s * 128 + D],
                        start=(b == b0),
                        stop=(b == b0 + nb - 1),
                    )
                    if b == b0 + nb - 1:
                        # q0 and q1 both land via the Scalar engine so the DVE
                        # stream stays free of psum round-trips
                        dst = S if q == 0 else S1
                        nc.scalar.copy(out=dst[:, j * D:(j + 1) * D], in_=ps[0][:])
                        if run_i == half_run_i:
                            half_cb()
                        run_i += 1
                end_cb()

            def merge_rec(jr, rec):
                """S += S1 then the step recurrence, on pair range jr."""
                c0, c1 = jr[0] * D, jr[1] * D
                nc.vector.tensor_tensor(
                    out=S[:, c0:c1], in0=S[:, c0:c1], in1=S1[:, c0:c1], op=ADD)
                rec(c0, c1)

            def writeback(k, src, half):
                """src pair-range (f32) -> Tc16 -> tsh -> AllGather -> tfull."""
                (j0, j1) = (0, JH) if half == 0 else (JH, PAIRS)
                nc.vector.tensor_copy(
                    out=T163[:, j0:j1, 0:D],
                    in_=src[:, j0 * D:j1 * D].rearrange(
                        "p (j f) -> p j f", j=j1 - j0))
                tsh = (tshA if half == 0 else tshB)[k]
                c0, c1 = j0 * PADC, j1 * PADC
                nc.sync.dma_start(out=tsh[:, :], in_=Tc16[:, c0:c1])
                if not no_cc:
                    nc.gpsimd.collective_compute(
                        "AllGather",
                        mybir.AluOpType.bypass,
                        ins=[tsh[:, :]],
                        outs=[(tfullA if half == 0 else tfullB)[k][:, :]],
                        replica_groups=rg,
                    )

            def proj(j0, j1):
                """out[j0:j1 pairs] = U @ W + bias (S reused as staging)."""
                for j in range(j0, j1):
                    pt = psp2.tile([128, 128], f32, tag="pt", name="pt")
                    nc.tensor.transpose(
                        out=pt[0:D, :], in_=U[:, j * D:(j + 1) * D],
                        identity=ident_t[:])
                    ut = up.tile([128, 128], f32, tag="ut", name="ut")
                    nc.scalar.copy(out=ut[0:D, :], in_=pt[0:D, :])
                    po = psp2.tile([128, D], f32, tag="po", name="po")
                    nc.tensor.matmul(
                        out=po[:], lhsT=ut[0:D, :], rhs=wmat_t[:, :],
                        start=True, stop=True)
                    nc.vector.tensor_tensor(
                        out=S[:, j * D:(j + 1) * D], in0=po[:], in1=bias_t[:],
                        op=ADD)
                    r1 = min((j + 1) * 128, SH)
                    eng = nc.sync if j % 2 == 0 else nc.scalar
                    eng.dma_start(
                        out=out_d[j * 128:r1, :],
                        in_=S[0:r1 - j * 128, j * D:(j + 1) * D],
                    )

            tA, tB = tbl0[0:ROWSA, :], tbl0[ROWSA:NPAD, :]

            # ---- k=1 : T1 = 2*spmm(H) - T0
            def rec1(c0, c1):
                nc.vector.scalar_tensor_tensor(
                    out=Tc[:, c0:c1], in0=S[:, c0:c1], scalar=2.0,
                    in1=Tp[:, c0:c1], op0=MUL, op1=SUB)
                nc.vector.tensor_tensor(
                    out=U[:, c0:c1], in0=U[:, c0:c1], in1=Tc[:, c0:c1], op=ADD)

            def half1():
                merge_rec((0, JH), rec1)
                if n_steps >= 2:
                    writeback(0, Tc, 0)

            def end1():
                merge_rec((JH, PAIRS), rec1)
                if n_steps >= 2:
                    writeback(0, Tc, 1)

            spmm(tA, tB, half1, end1)

            # ---- k>=2 : T_next = 2*(2*spmm(T) - T) - T_prev
            def rec_k(c0, c1):
                nc.vector.scalar_tensor_tensor(
                    out=S[:, c0:c1], in0=S[:, c0:c1], scalar=2.0,
                    in1=Tc[:, c0:c1], op0=MUL, op1=SUB)
                nc.vector.scalar_tensor_tensor(
                    out=Tp[:, c0:c1], in0=S[:, c0:c1], scalar=2.0,
                    in1=Tp[:, c0:c1], op0=MUL, op1=SUB)
                nc.vector.tensor_tensor(
                    out=U[:, c0:c1], in0=U[:, c0:c1], in1=Tp[:, c0:c1], op=ADD)

            if n_steps >= 2:
                # ---- k=2 : new T lands in the Tp tile; roles swap after
                def half2():
                    merge_rec((0, JH), rec_k)
                    if n_steps >= 3:
                        writeback(1, Tp, 0)

                def end2():
                    merge_rec((JH, PAIRS), rec_k)
                    if n_steps >= 3:
                        writeback(1, Tp, 1)

                tabA = tA if no_cc else tfullA[0][:, :]
                tabB = tB if no_cc else tfullB[0][:, :]
                spmm(tabA, tabB, half2, end2)
                Tp, Tc = Tc, Tp

            if n_steps >= 3:
                # ---- k=3 : final projection overlaps the B half
                def half3():
                    merge_rec((0, JH), rec_k)
                    if not no_final:
                        proj(0, JH)

                def end3():
                    merge_rec((JH, PAIRS), rec_k)
                    if not no_final:
                        proj(JH, PAIRS)

                tabA = tA if no_cc else tfullA[1][:, :]
                tabB = tB if no_cc else tfullB[1][:, :]
                spmm(tabA, tabB, half3, end3)
            elif not no_final:
                proj(0, PAIRS)

    nc.compile()
    return nc


def kernel(rows, cols, vals, H, W, bias):
    global last_results
    import os
    from concourse.bass_utils import run_bass_kernel_spmd

    H = np.asarray(H).astype(np.float32)
    W = np.asarray(W).astype(np.float32)
    bias = np.asarray(bias).astype(np.float32)

    # NOTE: dma_gather ucode hangs above 1024 indices per call -> W <= 8
    WW = _env_int("CHEB_W", 8)
    R = _env_int("CHEB_RING", 64)
    assert R % WW == 0

    TOTB, windows, runs, core_arrays = _preprocess(rows, cols, vals, WW)
    nc = _build_program(TOTB, windows, runs, WW, R)

    # bf16 node tables: half A rows [0, 25600), half B rows [25600, 50176)
    tbl = np.zeros((NPAD, PADC), BF16)
    qn, rown = _row_q_of_node(np.arange(N_NODES))
    tbl[np.where(qn == 0, rown, ROWSA + rown), :D] = H.astype(BF16)

    iota128 = np.broadcast_to(np.arange(128, dtype=np.float32), (128, 128))
    iota128 = iota128.astype(BF16)
    ident = np.eye(128, dtype=np.float32)
    biasb = np.broadcast_to(bias, (128, D)).copy()

    in_maps = []
    for c in range(C):
        widx, dloc_col, vals_col = core_arrays[c]
        # hshc: compact [128, 49*96] partition-major layout of this core's shard
        hshc = np.zeros((128, PAIRS, D), np.float32)
        hrows = H[c * SH:(c + 1) * SH]
        for j in range(PAIRS):
            r0, r1 = j * 128, min((j + 1) * 128, SH)
            hshc[0:r1 - r0, j, :] = hrows[r0:r1]
        in_maps.append({
            "tbl0": tbl,
            "hshc": hshc.reshape(128, PAIRS * D),
            "widx": widx,
            "dloc": dloc_col,
            "wval": vals_col,
            "iota128": iota128,
            "ident": ident,
            "wmat": W,
            "biasb": biasb,
        })

    res = run_bass_kernel_spmd(
        nc, in_maps, core_ids=list(range(C)),
        trace=bool(_env_int("CHEB_TRACE", 0)),
    )
    last_results = res
    return np.concatenate([res.results[c]["out"] for c in range(C)], axis=0)
